# revision 12
# baseline (speedup 1.0000x reference)
"""Trainium2 Bass kernel for nn_Encoding (dense transformer block with
inter-attention + gated fusion), data-parallel over batch on 8 NeuronCores.

Reference math per batch b (P: [n, d], weights small):
  wa, wb, wc = split(w_itr_att)
  A[i,j]   = P[i].wb + P[j].wa + sum_d P[i,d]*wc[d]*P[j,d]
  SA       = softmax_j(A)
  itr      = SA @ P
  Pc       = [P, itr]
  z = tanh(Pc@w1+b1); r = sig(Pc@w2+b2); f = sig(Pc@w3+b3)
  out      = r*P + f*z

Key tricks:
  - exp(P[i].wb) cancels between softmax numerator and denominator -> wb
    term dropped entirely.
  - Scores computed TRANSPOSED (At[j,i]) so P[j].wa is a per-partition
    exp() bias and both numerator (P^T @ T) and denominator (ones^T @ T)
    matmuls consume T=exp(At) in natural layout -- no [n,n] transpose.
  - Rows of P are processed in a fixed permutation (n = p*8+t instead of
    t*128+p) so each SBUF partition's DMA data is one contiguous 4KB run
    (128 descriptors per P load instead of 1024). Attention sums over all
    j and everything downstream uses the same permutation, so it cancels.
  - sigmoid(x) = 0.5 + 0.5*tanh(0.5*x) keeps all activations within the
    exp/tanh ACT table set (no table switches).
  - Matmuls run in bf16 (separate LDWEIGHTS pipelines behind MATMUL,
    unlike fp32/fp32r self-loading matmuls); accumulation is fp32 in
    PSUM; softmax/normalization arithmetic stays fp32.
"""
from contextlib import ExitStack

import numpy as np

import concourse.bass as bass
import concourse.mybir as mybir
import concourse.tile as tile
import concourse.tile_sem_assignment as tsa
from concourse import bacc
from concourse.bass_utils import run_bass_kernel_spmd
from concourse.masks import make_identity

# All HWDGE DMAs here are issued from the single SP sequencer (one physical
# FIFO ring -> in-order completion), so one completion semaphore suffices and
# keeps per-instruction sync-wait counts low.
tsa.NUM_HWDGE_SEMS = 1

B, N, D = 32, 1024, 128
NCORES = 8
BPC = B // NCORES          # batches per core
NB = N // 128              # 128-row blocks per batch
f32 = mybir.dt.float32
bf16 = mybir.dt.bfloat16
Exp = mybir.ActivationFunctionType.Exp
Tanh = mybir.ActivationFunctionType.Tanh


class _State:
    pass


def _phase_a(nc, s, bi):
    """Load P (permuted-contiguous), cast to bf16, build P^T, Pwc^T, v."""
    work, ps_itr, ps_dv = s.work, s.ps_itr, s.ps_dv
    pn = work.tile([128, NB, 128], f32, tag=f"pn")
    nc.sync.dma_start(out=pn, in_=s.P[bi].rearrange("(p t) d -> p t d", t=NB))
    pn_h = work.tile([128, NB, 128], bf16, tag=f"pn_h")
    nc.gpsimd.tensor_copy(pn_h, pn)

    pt_h = work.tile([128, NB, 128], bf16, tag="pt_h")   # [d, n']
    for half in range(2):
        tp_ps = ps_itr.tile([128, 512], bf16, tag="itr")
        for q in range(4):
            jb = half * 4 + q
            nc.tensor.transpose(tp_ps[:, q * 128:(q + 1) * 128],
                                pn_h[:, jb, :], s.ident_h)
        nc.vector.tensor_copy(pt_h[:, half * 4:(half + 1) * 4, :], tp_ps)

    pwct_h = work.tile([128, NB, 128], bf16, tag="pwct_h")  # wc[d]*P^T
    nc.vector.tensor_scalar_mul(pwct_h, pt_h, s.wc_col)

    # v[j] = P[j].wa per j-block into [128, NB, 2] (col 0 used)
    v_ps = ps_dv.tile([128, NB, 2], f32, tag="dv")
    for jb in range(NB):
        nc.tensor.matmul(v_ps[:, jb, :], pt_h[:, jb, :], s.wa_col,
                         start=True, stop=True)
    v_sb = work.tile([128, NB, 2], f32, tag="v_sb")
    nc.vector.tensor_copy(v_sb, v_ps)
    s.pn[bi], s.pn_h[bi], s.pt_h[bi] = pn, pn_h, pt_h
    s.pwct_h[bi], s.v_sb[bi] = pwct_h, v_sb


def _phase_b(nc, s, bi):
    """Scores At[j,i] + exp -> T (bf16)."""
    st = s.big.tile([128, NB, N], bf16, tag="st")
    pt_h, pwct_h, v_sb = s.pt_h[bi], s.pwct_h[bi], s.v_sb[bi]
    for jb in range(NB):
        at_ps = s.ps_at.tile([128, 1024], f32, tag="at")
        nc.tensor.matmul(at_ps[:, 0:512], pt_h[:, jb, :],
                         pwct_h[:, 0:4, :], start=True, stop=True)
        nc.tensor.matmul(at_ps[:, 512:1024], pt_h[:, jb, :],
                         pwct_h[:, 4:8, :], start=True, stop=True)
        nc.scalar.activation(st[:, jb, :], at_ps, Exp, bias=v_sb[:, jb, 0:1])
    s.st[bi] = st


def _phase_c(nc, s, bi):
    """numerator/denominator matmuls + normalization -> itr^T (bf16)."""
    work, ps_itr, ps_dv = s.work, s.ps_itr, s.ps_dv
    st, pn_h = s.st[bi], s.pn_h[bi]
    itrt_h = work.tile([128, NB, 128], bf16, tag="itrt_h")  # itr^T [d, n']
    for c in range(2):
        cs = slice(c * 512, (c + 1) * 512)
        itr_ps = ps_itr.tile([128, 512], f32, tag="itr")
        den_ps = ps_dv.tile([1, 512], f32, tag="dv")
        for jb in range(NB):
            nc.tensor.matmul(itr_ps, pn_h[:, jb, :], st[:, jb, cs],
                             start=(jb == 0), stop=(jb == NB - 1))
        for jb in range(NB):
            nc.tensor.matmul(den_ps, s.ones_col, st[:, jb, cs],
                             start=(jb == 0), stop=(jb == NB - 1))
        # broadcast raw denominator to all partitions via ones x den, then
        # fast reciprocal on [128,512] (lane-parallel) and multiply.
        den_row = work.tile([1, 512], bf16, tag="den_row")
        nc.vector.tensor_copy(den_row, den_ps)
        bc_ps = ps_dv.tile([128, 512], f32, tag="dv")
        nc.tensor.matmul(bc_ps, s.ones_row, den_row, start=True, stop=True)
        bc_sb = work.tile([128, 512], f32, tag="bc_sb")
        nc.vector.tensor_copy(bc_sb, bc_ps)
        recip_sb = work.tile([128, 512], f32, tag="recip_sb")
        nc.vector.reciprocal_approx_fast(recip_sb, bc_sb)
        with nc.allow_low_precision(reason="bf16 itr weights"):
            nc.vector.tensor_mul(itrt_h[:, c * 4:(c + 1) * 4, :],
                                 itr_ps, recip_sb)
    s.itrt_h[bi] = itrt_h


def _phase_d(nc, s, bi):
    """Gates + output, in two half-batches so ACT/DVE/DMA pipeline."""
    work, ps_dv = s.work, s.ps_dv
    pn, pt_h, itrt_h = s.pn[bi], s.pt_h[bi], s.itrt_h[bi]
    out_sb = work.tile([128, NB, 128], f32, tag="out_sb")
    for half in range(2):
        hb = slice(half * 4, (half + 1) * 4)
        gcat = work.tile([128, 4, 384], f32, tag="gcat")
        for q in range(4):
            ib = half * 4 + q
            g_ps = ps_dv.tile([128, 384], f32, tag="dv")
            nc.tensor.matmul(g_ps, pt_h[:, ib, :], s.w_top,
                             start=True, stop=False)
            nc.tensor.matmul(g_ps, itrt_h[:, ib, :], s.w_bot,
                             start=False, stop=False)
            nc.tensor.matmul(g_ps, s.ones_row, s.bcat,
                             start=False, stop=True)
            nc.vector.tensor_copy(gcat[:, q, :], g_ps)

        z_t = work.tile([128, 4, 128], f32, tag="z_t")
        nc.scalar.activation(z_t, gcat[:, :, 0:128], Tanh)
        rf_t = work.tile([128, 4, 256], f32, tag="rf_t")
        nc.scalar.activation(rf_t, gcat[:, :, 128:384], Tanh, scale=0.5)
        # r = 0.5 + 0.5*tanh(0.5 x), f likewise
        rf_a = work.tile([128, 4, 256], f32, tag="rf_a")
        nc.gpsimd.tensor_scalar(rf_a, rf_t, 0.5, 0.5,
                                mybir.AluOpType.mult, mybir.AluOpType.add)
        m1 = work.tile([128, 4, 128], f32, tag="m1")
        nc.gpsimd.tensor_mul(m1, rf_a[:, :, 0:128], pn[:, hb, :])   # r * P
        m2 = work.tile([128, 4, 128], f32, tag="m2")
        nc.vector.tensor_mul(m2, rf_a[:, :, 128:256], z_t)          # f * z
        nc.vector.tensor_add(out_sb[:, hb, :], m1, m2)

    nc.sync.dma_start(out=s.out[bi].rearrange("(p t) d -> p t d", t=NB),
                      in_=out_sb)


def _body(nc, tc, ctx):
    s = _State()
    s.P = nc.dram_tensor("P", [BPC, N, D], f32, kind="ExternalInput")
    w_att = nc.dram_tensor("w_itr_att", [3 * D], f32, kind="ExternalInput")
    w1 = nc.dram_tensor("w1", [2 * D, D], f32, kind="ExternalInput")
    w2 = nc.dram_tensor("w2", [2 * D, D], f32, kind="ExternalInput")
    w3 = nc.dram_tensor("w3", [2 * D, D], f32, kind="ExternalInput")
    b1 = nc.dram_tensor("b1", [D], f32, kind="ExternalInput")
    b2 = nc.dram_tensor("b2", [D], f32, kind="ExternalInput")
    b3 = nc.dram_tensor("b3", [D], f32, kind="ExternalInput")
    s.out = nc.dram_tensor("out", [BPC, N, D], f32, kind="ExternalOutput")

    singles = ctx.enter_context(tc.tile_pool(name="singles", bufs=1))
    s.work = ctx.enter_context(tc.tile_pool(name="work", bufs=2))
    s.big = ctx.enter_context(tc.tile_pool(name="big", bufs=2))
    s.ps_at = ctx.enter_context(tc.tile_pool(name="ps_at", bufs=2, space="PSUM"))
    s.ps_itr = ctx.enter_context(tc.tile_pool(name="ps_itr", bufs=2, space="PSUM"))
    s.ps_dv = ctx.enter_context(tc.tile_pool(name="ps_dv", bufs=2, space="PSUM"))
    s.pn, s.pn_h, s.pt_h, s.pwct_h = {}, {}, {}, {}
    s.v_sb, s.st, s.itrt_h = {}, {}, {}

    # ---- constants ----
    # w_itr_att as a single-descriptor row; wa/wc become per-partition
    # columns via tiny K=1 fp32 matmuls (exact: multiply by 1.0).
    watt_row = singles.tile([1, 3 * D], f32)
    nc.sync.dma_start(out=watt_row, in_=w_att.rearrange("(o c) -> o c", o=1))
    ones2_f = singles.tile([1, 2], f32)
    nc.vector.memset(ones2_f, 1.0)
    wcols_ps = s.ps_dv.tile([128, 2, 2], f32, tag="dv")
    nc.tensor.matmul(wcols_ps[:, 0, :], watt_row[:, 0:128], ones2_f,
                     start=True, stop=True)          # wa
    nc.tensor.matmul(wcols_ps[:, 1, :], watt_row[:, 256:384], ones2_f,
                     start=True, stop=True)          # wc
    s.wa_col = singles.tile([128, 2], bf16)
    nc.vector.tensor_copy(s.wa_col, wcols_ps[:, 0, :])
    s.wc_col = singles.tile([128, 1], f32)
    nc.vector.tensor_copy(s.wc_col, wcols_ps[:, 1, 0:1])

    ident = singles.tile([128, 128], f32)
    make_identity(nc, ident)
    s.ident_h = singles.tile([128, 128], bf16)
    nc.vector.tensor_copy(s.ident_h, ident)

    ones_f = singles.tile([128, 1], f32)
    nc.vector.memset(ones_f, 1.0)
    ones_rf = singles.tile([1, 128], f32)
    nc.vector.memset(ones_rf, 1.0)
    s.ones_col = singles.tile([128, 1], bf16)   # lhsT for denominator matmul
    nc.vector.tensor_copy(s.ones_col, ones_f)
    s.ones_row = singles.tile([1, 128], bf16)   # lhsT for broadcast matmuls
    nc.vector.tensor_copy(s.ones_row, ones_rf)

    # First batch's load goes ahead of the (descriptor-heavy) gate-weight
    # DMAs: gates need the weights only late in batch 0.
    _phase_a(nc, s, 0)

    # Gate weights: Wtop = rows 0:128 of [w1|w2|w3], Wbot = rows 128:256.
    wstage = singles.tile([128, 2, 3, 128], f32)
    for gi, w in enumerate((w1, w2, w3)):
        nc.sync.dma_start(out=wstage[:, 0, gi, :], in_=w[0:128, :])
        nc.sync.dma_start(out=wstage[:, 1, gi, :], in_=w[128:256, :])
    s.w_top = singles.tile([128, 384], bf16)
    s.w_bot = singles.tile([128, 384], bf16)
    nc.vector.tensor_copy(s.w_top, wstage[:, 0, :, :])
    nc.vector.tensor_copy(s.w_bot, wstage[:, 1, :, :])

    bstage = singles.tile([1, 3, 128], f32)
    for gi, bvec in enumerate((b1, b2, b3)):
        nc.sync.dma_start(out=bstage[:, gi, :],
                          in_=bvec.rearrange("(o p) -> o p", o=1))
    s.bcat = singles.tile([1, 384], bf16)
    nc.vector.tensor_copy(s.bcat, bstage)

    # Software pipeline: next batch's load/transpose work is emitted right
    # after this batch's score matmuls so the PE never starves at batch
    # boundaries.
    for bi in range(BPC):
        _phase_b(nc, s, bi)
        if bi + 1 < BPC:
            _phase_a(nc, s, bi + 1)
        _phase_c(nc, s, bi)
        _phase_d(nc, s, bi)


_NC_CACHE = {}


def _get_nc():
    if "nc" not in _NC_CACHE:
        nc = bacc.Bacc(None)
        with tile.TileContext(nc) as tc:
            with ExitStack() as ctx:
                _body(nc, tc, ctx)
        nc.finalize()
        _NC_CACHE["nc"] = nc
    return _NC_CACHE["nc"]


def _run(inputs, **kw):
    nc = _get_nc()
    in_maps = []
    for c in range(NCORES):
        m = {
            "P": np.ascontiguousarray(inputs["P"][c * BPC:(c + 1) * BPC]),
            "w_itr_att": np.asarray(inputs["w_itr_att"]),
            "w1": np.asarray(inputs["w1"]),
            "w2": np.asarray(inputs["w2"]),
            "w3": np.asarray(inputs["w3"]),
            "b1": np.asarray(inputs["b1"]),
            "b2": np.asarray(inputs["b2"]),
            "b3": np.asarray(inputs["b3"]),
        }
        in_maps.append({k: np.asarray(v, dtype=np.float32) for k, v in m.items()})
    res = run_bass_kernel_spmd(nc, in_maps, core_ids=list(range(NCORES)), **kw)
    outp = np.concatenate([r["out"] for r in res.results], axis=0)
    return outp.astype(np.float32), res


def kernel(**inputs):
    out, _ = _run(inputs)
    return out


# revision 16
# speedup vs baseline: 1.0385x; 1.0385x over previous
"""Trainium2 Bass kernel for nn_Encoding (dense transformer block with
inter-attention + gated fusion), data-parallel over batch on 8 NeuronCores.

Reference math per batch b (P: [n, d], weights small):
  wa, wb, wc = split(w_itr_att)
  A[i,j]   = P[i].wb + P[j].wa + sum_d P[i,d]*wc[d]*P[j,d]
  SA       = softmax_j(A)
  itr      = SA @ P
  Pc       = [P, itr]
  z = tanh(Pc@w1+b1); r = sig(Pc@w2+b2); f = sig(Pc@w3+b3)
  out      = r*P + f*z

Key tricks:
  - exp(P[i].wb) cancels between softmax numerator and denominator -> wb
    term dropped entirely.
  - Scores computed TRANSPOSED (At[j,i]) so P[j].wa is a per-partition
    exp() bias and both numerator (P^T @ T) and denominator (ones^T @ T)
    matmuls consume T=exp(At) in natural layout -- no [n,n] transpose.
  - Rows of P are processed in a fixed permutation (n = p*8+t instead of
    t*128+p) so each SBUF partition's DMA data is one contiguous 4KB run
    (128 descriptors per P load instead of 1024). Attention sums over all
    j and everything downstream uses the same permutation, so it cancels.
  - sigmoid(x) = 0.5 + 0.5*tanh(0.5*x) keeps all activations within the
    exp/tanh ACT table set (no table switches).
  - Matmuls run in bf16 (separate LDWEIGHTS pipelines behind MATMUL,
    unlike fp32/fp32r self-loading matmuls); accumulation is fp32 in
    PSUM; softmax/normalization arithmetic stays fp32.
"""
from contextlib import ExitStack

import numpy as np

import concourse.bass as bass
import concourse.mybir as mybir
import concourse.tile as tile
import concourse.tile_sem_assignment as tsa
from concourse import bacc
from concourse.bass_utils import run_bass_kernel_spmd
from concourse.masks import make_identity

# All HWDGE DMAs here are issued from the single SP sequencer (one physical
# FIFO ring -> in-order completion), so one completion semaphore suffices and
# keeps per-instruction sync-wait counts low.
tsa.NUM_HWDGE_SEMS = 1

B, N, D = 32, 1024, 128
NCORES = 8
BPC = B // NCORES          # batches per core
NB = N // 128              # 128-row blocks per batch
f32 = mybir.dt.float32
bf16 = mybir.dt.bfloat16
Exp = mybir.ActivationFunctionType.Exp
Tanh = mybir.ActivationFunctionType.Tanh


class _State:
    pass


def _load(nc, s, bi):
    """DMA P (permuted-contiguous rows: 128 x 4KB descriptors) + bf16 cast."""
    pn = s.work.tile([128, NB, 128], f32, tag="pn")
    nc.sync.dma_start(out=pn, in_=s.P[bi].rearrange("(p t) d -> p t d", t=NB))
    pn_h = s.work.tile([128, NB, 128], bf16, tag="pn_h")
    nc.gpsimd.tensor_copy(pn_h, pn)
    s.pn[bi], s.pn_h[bi] = pn, pn_h


def _prep(nc, s, bi):
    """Build P^T (PE transpose), Pwc^T, and v = P.wa (DVE fused mul+reduce)."""
    work, ps_itr = s.work, s.ps_itr
    pn, pn_h = s.pn[bi], s.pn_h[bi]
    pt_h = work.tile([128, NB, 128], bf16, tag="pt_h")   # [d, n']
    for half in range(2):
        tp_ps = ps_itr.tile([128, 512], bf16, tag="itr")
        for q in range(4):
            jb = half * 4 + q
            nc.tensor.transpose(tp_ps[:, q * 128:(q + 1) * 128],
                                pn_h[:, jb, :], s.ident_h)
        nc.vector.tensor_copy(pt_h[:, half * 4:(half + 1) * 4, :], tp_ps)

    pwct_h = work.tile([128, NB, 128], bf16, tag="pwct_h")  # wc[d]*P^T
    nc.vector.tensor_scalar_mul(pwct_h, pt_h, s.wc_col)

    # v[j] = P[j].wa on the DVE (keeps PE and the shared PSUM rings free)
    v_sb = work.tile([128, NB], f32, tag="v_sb")
    vscr = work.tile([128, 128], f32, tag="vscr")
    for jb in range(NB):
        nc.vector.tensor_mul(vscr, pn[:, jb, :], s.wa_b)
        nc.vector.reduce_sum(v_sb[:, jb:jb + 1], vscr,
                             axis=mybir.AxisListType.X)
    s.pt_h[bi], s.pwct_h[bi], s.v_sb[bi] = pt_h, pwct_h, v_sb


def _phase_b(nc, s, bi):
    """Scores At[j,i] + exp -> T (bf16)."""
    st = s.big.tile([128, NB, N], bf16, tag="st")
    pt_h, pwct_h, v_sb = s.pt_h[bi], s.pwct_h[bi], s.v_sb[bi]
    for jb in range(NB):
        at_ps = s.ps_at.tile([128, 1024], f32, tag="at")
        nc.tensor.matmul(at_ps[:, 0:512], pt_h[:, jb, :],
                         pwct_h[:, 0:4, :], start=True, stop=True)
        nc.tensor.matmul(at_ps[:, 512:1024], pt_h[:, jb, :],
                         pwct_h[:, 4:8, :], start=True, stop=True)
        nc.scalar.activation(st[:, jb, :], at_ps, Exp, bias=v_sb[:, jb:jb + 1])
    s.st[bi] = st


def _phase_c(nc, s, bi):
    """numerator/denominator matmuls + normalization -> itr^T (bf16)."""
    work, ps_itr, ps_dv = s.work, s.ps_itr, s.ps_dv
    st, pn_h = s.st[bi], s.pn_h[bi]
    itrt_h = work.tile([128, NB, 128], bf16, tag="itrt_h")  # itr^T [d, n']
    for c in range(2):
        cs = slice(c * 512, (c + 1) * 512)
        itr_ps = ps_itr.tile([128, 512], f32, tag="itr")
        den_ps = ps_dv.tile([1, 512], f32, tag="dv")
        for jb in range(NB):
            nc.tensor.matmul(itr_ps, pn_h[:, jb, :], st[:, jb, cs],
                             start=(jb == 0), stop=(jb == NB - 1))
        for jb in range(NB):
            nc.tensor.matmul(den_ps, s.ones_col, st[:, jb, cs],
                             start=(jb == 0), stop=(jb == NB - 1))
        # broadcast raw denominator to all partitions via ones x den, then
        # fast reciprocal on [128,512] (lane-parallel) and multiply.
        den_row = work.tile([1, 512], bf16, tag="den_row")
        nc.vector.tensor_copy(den_row, den_ps)
        bc_ps = ps_dv.tile([128, 512], f32, tag="dv")
        nc.tensor.matmul(bc_ps, s.ones_row, den_row, start=True, stop=True)
        bc_sb = work.tile([128, 512], f32, tag="bc_sb")
        nc.vector.tensor_copy(bc_sb, bc_ps)
        recip_sb = work.tile([128, 512], f32, tag="recip_sb")
        nc.vector.reciprocal_approx_fast(recip_sb, bc_sb)
        with nc.allow_low_precision(reason="bf16 itr weights"):
            nc.vector.tensor_mul(itrt_h[:, c * 4:(c + 1) * 4, :],
                                 itr_ps, recip_sb)
    s.itrt_h[bi] = itrt_h


def _phase_d(nc, s, bi):
    """Gates + output, in two half-batches so ACT/DVE/DMA pipeline."""
    work, ps_dv = s.work, s.ps_dv
    pn, pt_h, itrt_h = s.pn[bi], s.pt_h[bi], s.itrt_h[bi]
    out_sb = work.tile([128, NB, 128], f32, tag="out_sb")
    for half in range(2):
        hb = slice(half * 4, (half + 1) * 4)
        gcat = work.tile([128, 4, 384], f32, tag="gcat")
        for q in range(4):
            ib = half * 4 + q
            g_ps = ps_dv.tile([128, 384], f32, tag="dv")
            nc.tensor.matmul(g_ps, pt_h[:, ib, :], s.w_top,
                             start=True, stop=False)
            nc.tensor.matmul(g_ps, itrt_h[:, ib, :], s.w_bot,
                             start=False, stop=False)
            nc.tensor.matmul(g_ps, s.ones_row, s.bcat,
                             start=False, stop=True)
            nc.vector.tensor_copy(gcat[:, q, :], g_ps)

        z_t = work.tile([128, 4, 128], f32, tag="z_t")
        nc.scalar.activation(z_t, gcat[:, :, 0:128], Tanh)
        rf_t = work.tile([128, 4, 256], f32, tag="rf_t")
        nc.scalar.activation(rf_t, gcat[:, :, 128:384], Tanh, scale=0.5)
        # r = 0.5 + 0.5*tanh(0.5 x), f likewise
        rf_a = work.tile([128, 4, 256], f32, tag="rf_a")
        nc.gpsimd.tensor_scalar(rf_a, rf_t, 0.5, 0.5,
                                mybir.AluOpType.mult, mybir.AluOpType.add)
        m1 = work.tile([128, 4, 128], f32, tag="m1")
        nc.gpsimd.tensor_mul(m1, rf_a[:, :, 0:128], pn[:, hb, :])   # r * P
        m2 = work.tile([128, 4, 128], f32, tag="m2")
        nc.vector.tensor_mul(m2, rf_a[:, :, 128:256], z_t)          # f * z
        nc.vector.tensor_add(out_sb[:, hb, :], m1, m2)

    nc.sync.dma_start(out=s.out[bi].rearrange("(p t) d -> p t d", t=NB),
                      in_=out_sb)


def _body(nc, tc, ctx):
    s = _State()
    s.P = nc.dram_tensor("P", [BPC, N, D], f32, kind="ExternalInput")
    w_att = nc.dram_tensor("w_itr_att", [3 * D], f32, kind="ExternalInput")
    w1 = nc.dram_tensor("w1", [2 * D, D], f32, kind="ExternalInput")
    w2 = nc.dram_tensor("w2", [2 * D, D], f32, kind="ExternalInput")
    w3 = nc.dram_tensor("w3", [2 * D, D], f32, kind="ExternalInput")
    b1 = nc.dram_tensor("b1", [D], f32, kind="ExternalInput")
    b2 = nc.dram_tensor("b2", [D], f32, kind="ExternalInput")
    b3 = nc.dram_tensor("b3", [D], f32, kind="ExternalInput")
    s.out = nc.dram_tensor("out", [BPC, N, D], f32, kind="ExternalOutput")

    singles = ctx.enter_context(tc.tile_pool(name="singles", bufs=1))
    s.work = ctx.enter_context(tc.tile_pool(name="work", bufs=2))
    s.big = ctx.enter_context(tc.tile_pool(name="big", bufs=2))
    s.ps_at = ctx.enter_context(tc.tile_pool(name="ps_at", bufs=2, space="PSUM"))
    s.ps_itr = ctx.enter_context(tc.tile_pool(name="ps_itr", bufs=2, space="PSUM"))
    s.ps_dv = ctx.enter_context(tc.tile_pool(name="ps_dv", bufs=2, space="PSUM"))
    s.pn, s.pn_h, s.pt_h, s.pwct_h = {}, {}, {}, {}
    s.v_sb, s.st, s.itrt_h = {}, {}, {}

    # ---- constants ----
    # w_itr_att as a single-descriptor row; wa/wc become per-partition
    # columns via tiny K=1 fp32 matmuls (exact: multiply by 1.0).
    watt_row = singles.tile([1, 3 * D], f32)
    nc.sync.dma_start(out=watt_row, in_=w_att.rearrange("(o c) -> o c", o=1))
    ones2_f = singles.tile([1, 2], f32)
    nc.vector.memset(ones2_f, 1.0)
    wcols_ps = s.ps_dv.tile([128, 2], f32, tag="dv")
    nc.tensor.matmul(wcols_ps, watt_row[:, 256:384], ones2_f,
                     start=True, stop=True)          # wc
    s.wc_col = singles.tile([128, 1], f32)
    nc.vector.tensor_copy(s.wc_col, wcols_ps[:, 0:1])
    # wa replicated across all partitions via ones x wa_row (exact: x1.0)
    ones_rf0 = singles.tile([1, 128], f32)
    nc.vector.memset(ones_rf0, 1.0)
    wab_ps = s.ps_at.tile([128, 512], f32, tag="at")
    nc.tensor.matmul(wab_ps[:, 0:128], ones_rf0, watt_row[:, 0:128],
                     start=True, stop=True)
    s.wa_b = singles.tile([128, 128], f32)
    nc.vector.tensor_copy(s.wa_b, wab_ps[:, 0:128])

    ident = singles.tile([128, 128], f32)
    make_identity(nc, ident)
    s.ident_h = singles.tile([128, 128], bf16)
    nc.vector.tensor_copy(s.ident_h, ident)

    ones_f = singles.tile([128, 1], f32)
    nc.vector.memset(ones_f, 1.0)
    ones_rf = singles.tile([1, 128], f32)
    nc.vector.memset(ones_rf, 1.0)
    s.ones_col = singles.tile([128, 1], bf16)   # lhsT for denominator matmul
    nc.vector.tensor_copy(s.ones_col, ones_f)
    s.ones_row = singles.tile([1, 128], bf16)   # lhsT for broadcast matmuls
    nc.vector.tensor_copy(s.ones_row, ones_rf)

    _load(nc, s, 0)
    _prep(nc, s, 0)

    # Gate weights: Wtop = rows 0:128 of [w1|w2|w3], Wbot = rows 128:256.
    wstage = singles.tile([128, 2, 3, 128], f32)
    for gi, w in enumerate((w1, w2, w3)):
        nc.sync.dma_start(out=wstage[:, 0, gi, :], in_=w[0:128, :])
        nc.sync.dma_start(out=wstage[:, 1, gi, :], in_=w[128:256, :])
    s.w_top = singles.tile([128, 384], bf16)
    s.w_bot = singles.tile([128, 384], bf16)
    nc.vector.tensor_copy(s.w_top, wstage[:, 0, :, :])
    nc.vector.tensor_copy(s.w_bot, wstage[:, 1, :, :])

    bstage = singles.tile([1, 3, 128], f32)
    for gi, bvec in enumerate((b1, b2, b3)):
        nc.sync.dma_start(out=bstage[:, gi, :],
                          in_=bvec.rearrange("(o p) -> o p", o=1))
    s.bcat = singles.tile([1, 384], bf16)
    nc.vector.tensor_copy(s.bcat, bstage)

    # Software pipeline: next batch's load/transpose work is emitted right
    # after this batch's score matmuls so the PE never starves at batch
    # boundaries.
    for bi in range(BPC):
        if bi + 1 < BPC:
            _load(nc, s, bi + 1)
        _phase_b(nc, s, bi)
        if bi + 1 < BPC:
            _prep(nc, s, bi + 1)
        _phase_c(nc, s, bi)
        _phase_d(nc, s, bi)


_NC_CACHE = {}


def _get_nc():
    if "nc" not in _NC_CACHE:
        nc = bacc.Bacc(None)
        with tile.TileContext(nc) as tc:
            with ExitStack() as ctx:
                _body(nc, tc, ctx)
        nc.finalize()
        _NC_CACHE["nc"] = nc
    return _NC_CACHE["nc"]


def _run(inputs, **kw):
    nc = _get_nc()
    in_maps = []
    for c in range(NCORES):
        m = {
            "P": np.ascontiguousarray(inputs["P"][c * BPC:(c + 1) * BPC]),
            "w_itr_att": np.asarray(inputs["w_itr_att"]),
            "w1": np.asarray(inputs["w1"]),
            "w2": np.asarray(inputs["w2"]),
            "w3": np.asarray(inputs["w3"]),
            "b1": np.asarray(inputs["b1"]),
            "b2": np.asarray(inputs["b2"]),
            "b3": np.asarray(inputs["b3"]),
        }
        in_maps.append({k: np.asarray(v, dtype=np.float32) for k, v in m.items()})
    res = run_bass_kernel_spmd(nc, in_maps, core_ids=list(range(NCORES)), **kw)
    outp = np.concatenate([r["out"] for r in res.results], axis=0)
    return outp.astype(np.float32), res


def kernel(**inputs):
    out, _ = _run(inputs)
    return out


# revision 17
# speedup vs baseline: 1.1621x; 1.1190x over previous
"""Trainium2 Bass kernel for nn_Encoding (dense transformer block with
inter-attention + gated fusion), data-parallel over batch on 8 NeuronCores.

Reference math per batch b (P: [n, d], weights small):
  wa, wb, wc = split(w_itr_att)
  A[i,j]   = P[i].wb + P[j].wa + sum_d P[i,d]*wc[d]*P[j,d]
  SA       = softmax_j(A)
  itr      = SA @ P
  Pc       = [P, itr]
  z = tanh(Pc@w1+b1); r = sig(Pc@w2+b2); f = sig(Pc@w3+b3)
  out      = r*P + f*z

Key tricks:
  - exp(P[i].wb) cancels between softmax numerator and denominator -> wb
    term dropped entirely.
  - Scores computed TRANSPOSED (At[j,i]) so P[j].wa is a per-partition
    exp() bias and both numerator (P^T @ T) and denominator (ones^T @ T)
    matmuls consume T=exp(At) in natural layout -- no [n,n] transpose.
  - Rows of P are processed in a fixed permutation (n = p*8+t instead of
    t*128+p) so each SBUF partition's DMA data is one contiguous 4KB run
    (128 descriptors per P load instead of 1024). Attention sums over all
    j and everything downstream uses the same permutation, so it cancels.
  - sigmoid(x) = 0.5 + 0.5*tanh(0.5*x) keeps all activations within the
    exp/tanh ACT table set (no table switches).
  - Matmuls run in bf16 (separate LDWEIGHTS pipelines behind MATMUL,
    unlike fp32/fp32r self-loading matmuls); accumulation is fp32 in
    PSUM; softmax/normalization arithmetic stays fp32.
"""
from contextlib import ExitStack

import numpy as np

import concourse.bass as bass
import concourse.mybir as mybir
import concourse.tile as tile
import concourse.tile_sem_assignment as tsa
from concourse import bacc
from concourse.bass_utils import run_bass_kernel_spmd
from concourse.masks import make_identity

# All HWDGE DMAs here are issued from the single SP sequencer (one physical
# FIFO ring -> in-order completion), so one completion semaphore suffices and
# keeps per-instruction sync-wait counts low.
tsa.NUM_HWDGE_SEMS = 1

B, N, D = 32, 1024, 128
NCORES = 8
BPC = B // NCORES          # batches per core
NB = N // 128              # 128-row blocks per batch
f32 = mybir.dt.float32
bf16 = mybir.dt.bfloat16
Exp = mybir.ActivationFunctionType.Exp
Tanh = mybir.ActivationFunctionType.Tanh


class _State:
    pass


def _load(nc, s, bi):
    """DMA P (permuted-contiguous rows: 128 x 4KB descriptors) + bf16 cast."""
    pn = s.work.tile([128, NB, 128], f32, tag="pn")
    nc.sync.dma_start(out=pn, in_=s.P[bi].rearrange("(p t) d -> p t d", t=NB))
    pn_h = s.work.tile([128, NB, 128], bf16, tag="pn_h")
    nc.gpsimd.tensor_copy(pn_h, pn)
    s.pn[bi], s.pn_h[bi] = pn, pn_h


def _prep(nc, s, bi):
    """Build P^T (PE transpose), Pwc^T, and v = P.wa (DVE fused mul+reduce)."""
    work, ps_itr = s.work, s.ps_itr
    pn, pn_h = s.pn[bi], s.pn_h[bi]
    pt_h = work.tile([128, NB, 128], bf16, tag="pt_h")   # [d, n']
    for half in range(2):
        tp_ps = ps_itr.tile([128, 512], bf16, tag="itr")
        for q in range(4):
            jb = half * 4 + q
            nc.tensor.transpose(tp_ps[:, q * 128:(q + 1) * 128],
                                pn_h[:, jb, :], s.ident_h)
        nc.vector.tensor_copy(pt_h[:, half * 4:(half + 1) * 4, :], tp_ps)

    pwct_h = work.tile([128, NB, 128], bf16, tag="pwct_h")  # wc[d]*P^T
    nc.vector.tensor_scalar_mul(pwct_h, pt_h, s.wc_col)

    # v[j] = P[j].wa on the DVE (keeps PE and the shared PSUM rings free)
    v_sb = work.tile([128, NB], f32, tag="v_sb")
    vscr = work.tile([128, 128], f32, tag="vscr")
    for jb in range(NB):
        nc.vector.tensor_mul(vscr, pn[:, jb, :], s.wa_b)
        nc.vector.reduce_sum(v_sb[:, jb:jb + 1], vscr,
                             axis=mybir.AxisListType.X)
    s.pt_h[bi], s.pwct_h[bi], s.v_sb[bi] = pt_h, pwct_h, v_sb


def _phase_b(nc, s, bi):
    """Scores At[j,i] + exp -> T (bf16)."""
    st = s.big.tile([128, NB, N], bf16, tag="st")
    pt_h, pwct_h, v_sb = s.pt_h[bi], s.pwct_h[bi], s.v_sb[bi]
    for jb in range(NB):
        at_ps = s.ps_at.tile([128, 1024], f32, tag="at")
        nc.tensor.matmul(at_ps[:, 0:512], pt_h[:, jb, :],
                         pwct_h[:, 0:4, :], start=True, stop=True)
        nc.tensor.matmul(at_ps[:, 512:1024], pt_h[:, jb, :],
                         pwct_h[:, 4:8, :], start=True, stop=True)
        nc.scalar.activation(st[:, jb, :], at_ps, Exp, bias=v_sb[:, jb:jb + 1])
    s.st[bi] = st


def _phase_c(nc, s, bi):
    """numerator/denominator matmuls + normalization -> itr^T (bf16)."""
    work, ps_itr, ps_dv = s.work, s.ps_itr, s.ps_dv
    st, pn_h = s.st[bi], s.pn_h[bi]
    itrt_h = work.tile([128, NB, 128], bf16, tag="itrt_h")  # itr^T [d, n']
    for c in range(2):
        cs = slice(c * 512, (c + 1) * 512)
        itr_ps = ps_itr.tile([128, 512], f32, tag="itr")
        den_ps = ps_dv.tile([1, 512], f32, tag="dv")
        for jb in range(NB):
            nc.tensor.matmul(itr_ps, pn_h[:, jb, :], st[:, jb, cs],
                             start=(jb == 0), stop=(jb == NB - 1))
        for jb in range(NB):
            nc.tensor.matmul(den_ps, s.ones_col, st[:, jb, cs],
                             start=(jb == 0), stop=(jb == NB - 1))
        # broadcast raw denominator to all partitions via ones x den, then
        # fast reciprocal on [128,512] (lane-parallel) and multiply.
        den_row = work.tile([1, 512], bf16, tag="den_row")
        nc.vector.tensor_copy(den_row, den_ps)
        bc_ps = ps_dv.tile([128, 512], f32, tag="dv")
        nc.tensor.matmul(bc_ps, s.ones_row, den_row, start=True, stop=True)
        bc_sb = work.tile([128, 512], f32, tag="bc_sb")
        nc.vector.tensor_copy(bc_sb, bc_ps)
        recip_sb = work.tile([128, 512], f32, tag="recip_sb")
        nc.vector.reciprocal_approx_fast(recip_sb, bc_sb)
        with nc.allow_low_precision(reason="bf16 itr weights"):
            nc.vector.tensor_mul(itrt_h[:, c * 4:(c + 1) * 4, :],
                                 itr_ps, recip_sb)
    s.itrt_h[bi] = itrt_h


def _phase_d(nc, s, bi):
    """Gates + output, in two half-batches so ACT/DVE/DMA pipeline."""
    work, ps_dv = s.work, s.ps_dv
    pn, pt_h, itrt_h = s.pn[bi], s.pt_h[bi], s.itrt_h[bi]
    out_sb = work.tile([128, NB, 128], f32, tag="out_sb")
    for half in range(2):
        hb = slice(half * 4, (half + 1) * 4)
        gcat = work.tile([128, 4, 384], f32, tag="gcat")
        for q in range(4):
            ib = half * 4 + q
            g_ps = ps_dv.tile([128, 384], f32, tag="dv")
            nc.tensor.matmul(g_ps, pt_h[:, ib, :], s.w_top,
                             start=True, stop=False)
            nc.tensor.matmul(g_ps, itrt_h[:, ib, :], s.w_bot,
                             start=False, stop=False)
            nc.tensor.matmul(g_ps, s.ones_row, s.bcat,
                             start=False, stop=True)
            nc.vector.tensor_copy(gcat[:, q, :], g_ps)

        z_t = work.tile([128, 4, 128], f32, tag="z_t")
        nc.scalar.activation(z_t, gcat[:, :, 0:128], Tanh)
        rf_t = work.tile([128, 4, 256], f32, tag="rf_t")
        nc.scalar.activation(rf_t, gcat[:, :, 128:384], Tanh, scale=0.5)
        # r = 0.5 + 0.5*tanh(0.5 x), f likewise
        rf_a = work.tile([128, 4, 256], f32, tag="rf_a")
        nc.gpsimd.tensor_scalar(rf_a, rf_t, 0.5, 0.5,
                                mybir.AluOpType.mult, mybir.AluOpType.add)
        m1 = work.tile([128, 4, 128], f32, tag="m1")
        nc.gpsimd.tensor_mul(m1, rf_a[:, :, 0:128], pn[:, hb, :])   # r * P
        m2 = work.tile([128, 4, 128], f32, tag="m2")
        nc.vector.tensor_mul(m2, rf_a[:, :, 128:256], z_t)          # f * z
        nc.vector.tensor_add(out_sb[:, hb, :], m1, m2)

    nc.sync.dma_start(out=s.out[bi].rearrange("(p t) d -> p t d", t=NB),
                      in_=out_sb)


def _body(nc, tc, ctx):
    s = _State()
    s.P = nc.dram_tensor("P", [BPC, N, D], f32, kind="ExternalInput")
    w_att = nc.dram_tensor("w_itr_att", [3 * D], f32, kind="ExternalInput")
    w1 = nc.dram_tensor("w1", [2 * D, D], f32, kind="ExternalInput")
    w2 = nc.dram_tensor("w2", [2 * D, D], f32, kind="ExternalInput")
    w3 = nc.dram_tensor("w3", [2 * D, D], f32, kind="ExternalInput")
    b1 = nc.dram_tensor("b1", [D], f32, kind="ExternalInput")
    b2 = nc.dram_tensor("b2", [D], f32, kind="ExternalInput")
    b3 = nc.dram_tensor("b3", [D], f32, kind="ExternalInput")
    s.out = nc.dram_tensor("out", [BPC, N, D], f32, kind="ExternalOutput")

    singles = ctx.enter_context(tc.tile_pool(name="singles", bufs=1))
    s.work = ctx.enter_context(tc.tile_pool(name="work", bufs=2))
    s.big = ctx.enter_context(tc.tile_pool(name="big", bufs=2))
    s.ps_at = ctx.enter_context(tc.tile_pool(name="ps_at", bufs=2, space="PSUM"))
    s.ps_itr = ctx.enter_context(tc.tile_pool(name="ps_itr", bufs=2, space="PSUM"))
    s.ps_dv = ctx.enter_context(tc.tile_pool(name="ps_dv", bufs=2, space="PSUM"))
    s.pn, s.pn_h, s.pt_h, s.pwct_h = {}, {}, {}, {}
    s.v_sb, s.st, s.itrt_h = {}, {}, {}

    # ---- constants ----
    # w_itr_att as a single-descriptor row; wa/wc become per-partition
    # columns via tiny K=1 fp32 matmuls (exact: multiply by 1.0).
    watt_row = singles.tile([1, 3 * D], f32)
    nc.sync.dma_start(out=watt_row, in_=w_att.rearrange("(o c) -> o c", o=1))
    ones2_f = singles.tile([1, 2], f32)
    nc.vector.memset(ones2_f, 1.0)
    wcols_ps = s.ps_dv.tile([128, 2], f32, tag="dv")
    nc.tensor.matmul(wcols_ps, watt_row[:, 256:384], ones2_f,
                     start=True, stop=True)          # wc
    s.wc_col = singles.tile([128, 1], f32)
    nc.vector.tensor_copy(s.wc_col, wcols_ps[:, 0:1])
    # wa replicated across all partitions via ones x wa_row (exact: x1.0)
    ones_rf0 = singles.tile([1, 128], f32)
    nc.vector.memset(ones_rf0, 1.0)
    wab_ps = s.ps_at.tile([128, 512], f32, tag="at")
    nc.tensor.matmul(wab_ps[:, 0:128], ones_rf0, watt_row[:, 0:128],
                     start=True, stop=True)
    s.wa_b = singles.tile([128, 128], f32)
    nc.vector.tensor_copy(s.wa_b, wab_ps[:, 0:128])

    ident = singles.tile([128, 128], f32)
    make_identity(nc, ident)
    s.ident_h = singles.tile([128, 128], bf16)
    nc.vector.tensor_copy(s.ident_h, ident)

    ones_f = singles.tile([128, 1], f32)
    nc.vector.memset(ones_f, 1.0)
    ones_rf = singles.tile([1, 128], f32)
    nc.vector.memset(ones_rf, 1.0)
    s.ones_col = singles.tile([128, 1], bf16)   # lhsT for denominator matmul
    nc.vector.tensor_copy(s.ones_col, ones_f)
    s.ones_row = singles.tile([1, 128], bf16)   # lhsT for broadcast matmuls
    nc.vector.tensor_copy(s.ones_row, ones_rf)

    _load(nc, s, 0)
    _prep(nc, s, 0)

    # Gate weights: Wtop = rows 0:128 of [w1|w2|w3], Wbot = rows 128:256.
    wstage = singles.tile([128, 2, 3, 128], f32)
    for gi, w in enumerate((w1, w2, w3)):
        nc.gpsimd.dma_start(out=wstage[:, 0, gi, :], in_=w[0:128, :])
        nc.gpsimd.dma_start(out=wstage[:, 1, gi, :], in_=w[128:256, :])
    s.w_top = singles.tile([128, 384], bf16)
    s.w_bot = singles.tile([128, 384], bf16)
    nc.vector.tensor_copy(s.w_top, wstage[:, 0, :, :])
    nc.vector.tensor_copy(s.w_bot, wstage[:, 1, :, :])

    bstage = singles.tile([1, 3, 128], f32)
    for gi, bvec in enumerate((b1, b2, b3)):
        nc.gpsimd.dma_start(out=bstage[:, gi, :],
                          in_=bvec.rearrange("(o p) -> o p", o=1))
    s.bcat = singles.tile([1, 384], bf16)
    nc.vector.tensor_copy(s.bcat, bstage)

    # Software pipeline: next batch's load/transpose work is emitted right
    # after this batch's score matmuls so the PE never starves at batch
    # boundaries.
    for bi in range(BPC):
        if bi + 1 < BPC:
            _load(nc, s, bi + 1)
        _phase_b(nc, s, bi)
        if bi + 1 < BPC:
            _prep(nc, s, bi + 1)
        _phase_c(nc, s, bi)
        _phase_d(nc, s, bi)


_NC_CACHE = {}


def _get_nc():
    if "nc" not in _NC_CACHE:
        nc = bacc.Bacc(None)
        with tile.TileContext(nc) as tc:
            with ExitStack() as ctx:
                _body(nc, tc, ctx)
        nc.finalize()
        _NC_CACHE["nc"] = nc
    return _NC_CACHE["nc"]


def _run(inputs, **kw):
    nc = _get_nc()
    in_maps = []
    for c in range(NCORES):
        m = {
            "P": np.ascontiguousarray(inputs["P"][c * BPC:(c + 1) * BPC]),
            "w_itr_att": np.asarray(inputs["w_itr_att"]),
            "w1": np.asarray(inputs["w1"]),
            "w2": np.asarray(inputs["w2"]),
            "w3": np.asarray(inputs["w3"]),
            "b1": np.asarray(inputs["b1"]),
            "b2": np.asarray(inputs["b2"]),
            "b3": np.asarray(inputs["b3"]),
        }
        in_maps.append({k: np.asarray(v, dtype=np.float32) for k, v in m.items()})
    res = run_bass_kernel_spmd(nc, in_maps, core_ids=list(range(NCORES)), **kw)
    outp = np.concatenate([r["out"] for r in res.results], axis=0)
    return outp.astype(np.float32), res


def kernel(**inputs):
    out, _ = _run(inputs)
    return out


# revision 18
# speedup vs baseline: 1.2882x; 1.1086x over previous
"""Trainium2 Bass kernel for nn_Encoding (dense transformer block with
inter-attention + gated fusion), data-parallel over batch on 8 NeuronCores.

Reference math per batch b (P: [n, d], weights small):
  wa, wb, wc = split(w_itr_att)
  A[i,j]   = P[i].wb + P[j].wa + sum_d P[i,d]*wc[d]*P[j,d]
  SA       = softmax_j(A)
  itr      = SA @ P
  Pc       = [P, itr]
  z = tanh(Pc@w1+b1); r = sig(Pc@w2+b2); f = sig(Pc@w3+b3)
  out      = r*P + f*z

Key tricks:
  - exp(P[i].wb) cancels between softmax numerator and denominator -> wb
    term dropped entirely.
  - Scores computed TRANSPOSED (At[j,i]) so P[j].wa is a per-partition
    exp() bias and both numerator (P^T @ T) and denominator (ones^T @ T)
    matmuls consume T=exp(At) in natural layout -- no [n,n] transpose.
  - Rows of P are processed in a fixed permutation (n = p*8+t instead of
    t*128+p) so each SBUF partition's DMA data is one contiguous 4KB run
    (128 descriptors per P load instead of 1024). Attention sums over all
    j and everything downstream uses the same permutation, so it cancels.
  - sigmoid(x) = 0.5 + 0.5*tanh(0.5*x) keeps all activations within the
    exp/tanh ACT table set (no table switches).
  - Matmuls run in bf16 (separate LDWEIGHTS pipelines behind MATMUL,
    unlike fp32/fp32r self-loading matmuls); accumulation is fp32 in
    PSUM; softmax/normalization arithmetic stays fp32.
"""
from contextlib import ExitStack

import numpy as np

import concourse.bass as bass
import concourse.mybir as mybir
import concourse.tile as tile
import concourse.tile_sem_assignment as tsa
from concourse import bacc
from concourse.bass_utils import run_bass_kernel_spmd
from concourse.masks import make_identity

# All HWDGE DMAs here are issued from the single SP sequencer (one physical
# FIFO ring -> in-order completion), so one completion semaphore suffices and
# keeps per-instruction sync-wait counts low.
tsa.NUM_HWDGE_SEMS = 1

B, N, D = 32, 1024, 128
NCORES = 8
BPC = B // NCORES          # batches per core
NB = N // 128              # 128-row blocks per batch
f32 = mybir.dt.float32
bf16 = mybir.dt.bfloat16
Exp = mybir.ActivationFunctionType.Exp
Tanh = mybir.ActivationFunctionType.Tanh


class _State:
    pass


def _load(nc, s, bi):
    """DMA P (permuted-contiguous rows: 128 x 4KB descriptors) + bf16 cast."""
    pn = s.work.tile([128, NB, 128], f32, tag="pn")
    nc.sync.dma_start(out=pn, in_=s.P[bi].rearrange("(p t) d -> p t d", t=NB))
    pn_h = s.work.tile([128, NB, 128], bf16, tag="pn_h")
    nc.vector.tensor_copy(pn_h, pn)
    s.pn[bi], s.pn_h[bi] = pn, pn_h


def _prep(nc, s, bi):
    """Build P^T (PE transpose), Pwc^T, and v = P.wa (DVE fused mul+reduce)."""
    work, ps_itr = s.work, s.ps_itr
    pn, pn_h = s.pn[bi], s.pn_h[bi]
    pt_h = work.tile([128, NB, 128], bf16, tag="pt_h")   # [d, n']
    for half in range(2):
        tp_ps = ps_itr.tile([128, 512], bf16, tag="itr")
        for q in range(4):
            jb = half * 4 + q
            nc.tensor.transpose(tp_ps[:, q * 128:(q + 1) * 128],
                                pn_h[:, jb, :], s.ident_h)
        nc.vector.tensor_copy(pt_h[:, half * 4:(half + 1) * 4, :], tp_ps)

    pwct_h = work.tile([128, NB, 128], bf16, tag="pwct_h")  # wc[d]*P^T
    nc.vector.tensor_scalar_mul(pwct_h, pt_h, s.wc_col)

    # v[j] = P[j].wa on the DVE (keeps PE and the shared PSUM rings free)
    v_sb = work.tile([128, NB], f32, tag="v_sb")
    vscr = work.tile([128, 128], f32, tag="vscr")
    for jb in range(NB):
        nc.vector.tensor_mul(vscr, pn[:, jb, :], s.wa_b)
        nc.vector.reduce_sum(v_sb[:, jb:jb + 1], vscr,
                             axis=mybir.AxisListType.X)
    s.pt_h[bi], s.pwct_h[bi], s.v_sb[bi] = pt_h, pwct_h, v_sb


def _phase_b(nc, s, bi):
    """Scores At[j,i] + exp -> T (bf16)."""
    st = s.big.tile([128, NB, N], bf16, tag="st")
    pt_h, pwct_h, v_sb = s.pt_h[bi], s.pwct_h[bi], s.v_sb[bi]
    for jb in range(NB):
        at_ps = s.ps_at.tile([128, 1024], f32, tag="at")
        nc.tensor.matmul(at_ps[:, 0:512], pt_h[:, jb, :],
                         pwct_h[:, 0:4, :], start=True, stop=True)
        nc.tensor.matmul(at_ps[:, 512:1024], pt_h[:, jb, :],
                         pwct_h[:, 4:8, :], start=True, stop=True)
        nc.scalar.activation(st[:, jb, :], at_ps, Exp, bias=v_sb[:, jb:jb + 1])
    s.st[bi] = st


def _phase_c(nc, s, bi):
    """numerator/denominator matmuls + normalization -> itr^T (bf16)."""
    work, ps_itr, ps_dv = s.work, s.ps_itr, s.ps_dv
    st, pn_h = s.st[bi], s.pn_h[bi]
    itrt_h = work.tile([128, NB, 128], bf16, tag="itrt_h")  # itr^T [d, n']
    for c in range(2):
        cs = slice(c * 512, (c + 1) * 512)
        itr_ps = ps_itr.tile([128, 512], f32, tag="itr")
        den_ps = ps_dv.tile([1, 512], f32, tag="dv")
        for jb in range(NB):
            nc.tensor.matmul(itr_ps, pn_h[:, jb, :], st[:, jb, cs],
                             start=(jb == 0), stop=(jb == NB - 1))
        for jb in range(NB):
            nc.tensor.matmul(den_ps, s.ones_col, st[:, jb, cs],
                             start=(jb == 0), stop=(jb == NB - 1))
        # broadcast raw denominator to all partitions via ones x den, then
        # fast reciprocal on [128,512] (lane-parallel) and multiply.
        den_row = work.tile([1, 512], bf16, tag="den_row")
        nc.vector.tensor_copy(den_row, den_ps)
        bc_ps = ps_dv.tile([128, 512], f32, tag="dv")
        nc.tensor.matmul(bc_ps, s.ones_row, den_row, start=True, stop=True)
        bc_sb = work.tile([128, 512], f32, tag="bc_sb")
        nc.vector.tensor_copy(bc_sb, bc_ps)
        recip_sb = work.tile([128, 512], f32, tag="recip_sb")
        nc.vector.reciprocal_approx_fast(recip_sb, bc_sb)
        with nc.allow_low_precision(reason="bf16 itr weights"):
            nc.vector.tensor_mul(itrt_h[:, c * 4:(c + 1) * 4, :],
                                 itr_ps, recip_sb)
    s.itrt_h[bi] = itrt_h


def _phase_d(nc, s, bi):
    """Gates + output, in two half-batches so ACT/DVE/DMA pipeline."""
    work, ps_dv = s.work, s.ps_dv
    pn, pt_h, itrt_h = s.pn[bi], s.pt_h[bi], s.itrt_h[bi]
    out_sb = work.tile([128, NB, 128], f32, tag="out_sb")
    for half in range(2):
        hb = slice(half * 4, (half + 1) * 4)
        gcat = work.tile([128, 4, 384], f32, tag="gcat")
        for q in range(4):
            ib = half * 4 + q
            g_ps = ps_dv.tile([128, 384], f32, tag="dv")
            nc.tensor.matmul(g_ps, pt_h[:, ib, :], s.w_top,
                             start=True, stop=False)
            nc.tensor.matmul(g_ps, itrt_h[:, ib, :], s.w_bot,
                             start=False, stop=False)
            nc.tensor.matmul(g_ps, s.ones_row, s.bcat,
                             start=False, stop=True)
            nc.vector.tensor_copy(gcat[:, q, :], g_ps)

        z_t = work.tile([128, 4, 128], f32, tag="z_t")
        nc.scalar.activation(z_t, gcat[:, :, 0:128], Tanh)
        rf_t = work.tile([128, 4, 256], f32, tag="rf_t")
        nc.scalar.activation(rf_t, gcat[:, :, 128:384], Tanh, scale=0.5)
        # r = 0.5 + 0.5*tanh(0.5 x), f likewise
        rf_a = work.tile([128, 4, 256], f32, tag="rf_a")
        nc.gpsimd.tensor_scalar(rf_a, rf_t, 0.5, 0.5,
                                mybir.AluOpType.mult, mybir.AluOpType.add)
        m1 = work.tile([128, 4, 128], f32, tag="m1")
        nc.gpsimd.tensor_mul(m1, rf_a[:, :, 0:128], pn[:, hb, :])   # r * P
        m2 = work.tile([128, 4, 128], f32, tag="m2")
        nc.vector.tensor_mul(m2, rf_a[:, :, 128:256], z_t)          # f * z
        nc.vector.tensor_add(out_sb[:, hb, :], m1, m2)

    nc.sync.dma_start(out=s.out[bi].rearrange("(p t) d -> p t d", t=NB),
                      in_=out_sb)


def _body(nc, tc, ctx):
    s = _State()
    s.P = nc.dram_tensor("P", [BPC, N, D], f32, kind="ExternalInput")
    w_att = nc.dram_tensor("w_itr_att", [3 * D], f32, kind="ExternalInput")
    w1 = nc.dram_tensor("w1", [2 * D, D], f32, kind="ExternalInput")
    w2 = nc.dram_tensor("w2", [2 * D, D], f32, kind="ExternalInput")
    w3 = nc.dram_tensor("w3", [2 * D, D], f32, kind="ExternalInput")
    b1 = nc.dram_tensor("b1", [D], f32, kind="ExternalInput")
    b2 = nc.dram_tensor("b2", [D], f32, kind="ExternalInput")
    b3 = nc.dram_tensor("b3", [D], f32, kind="ExternalInput")
    s.out = nc.dram_tensor("out", [BPC, N, D], f32, kind="ExternalOutput")

    singles = ctx.enter_context(tc.tile_pool(name="singles", bufs=1))
    s.work = ctx.enter_context(tc.tile_pool(name="work", bufs=2))
    s.big = ctx.enter_context(tc.tile_pool(name="big", bufs=2))
    s.ps_at = ctx.enter_context(tc.tile_pool(name="ps_at", bufs=2, space="PSUM"))
    s.ps_itr = ctx.enter_context(tc.tile_pool(name="ps_itr", bufs=2, space="PSUM"))
    s.ps_dv = ctx.enter_context(tc.tile_pool(name="ps_dv", bufs=2, space="PSUM"))
    s.pn, s.pn_h, s.pt_h, s.pwct_h = {}, {}, {}, {}
    s.v_sb, s.st, s.itrt_h = {}, {}, {}

    # ---- constants ----
    # w_itr_att as a single-descriptor row; wa/wc become per-partition
    # columns via tiny K=1 fp32 matmuls (exact: multiply by 1.0).
    watt_row = singles.tile([1, 3 * D], f32)
    nc.sync.dma_start(out=watt_row, in_=w_att.rearrange("(o c) -> o c", o=1))
    ones2_f = singles.tile([1, 2], f32)
    nc.vector.memset(ones2_f, 1.0)
    wcols_ps = s.ps_dv.tile([128, 2], f32, tag="dv")
    nc.tensor.matmul(wcols_ps, watt_row[:, 256:384], ones2_f,
                     start=True, stop=True)          # wc
    s.wc_col = singles.tile([128, 1], f32)
    nc.vector.tensor_copy(s.wc_col, wcols_ps[:, 0:1])
    # wa replicated across all partitions via ones x wa_row (exact: x1.0)
    ones_rf0 = singles.tile([1, 128], f32)
    nc.vector.memset(ones_rf0, 1.0)
    wab_ps = s.ps_at.tile([128, 512], f32, tag="at")
    nc.tensor.matmul(wab_ps[:, 0:128], ones_rf0, watt_row[:, 0:128],
                     start=True, stop=True)
    s.wa_b = singles.tile([128, 128], f32)
    nc.vector.tensor_copy(s.wa_b, wab_ps[:, 0:128])

    ident = singles.tile([128, 128], f32)
    make_identity(nc, ident)
    s.ident_h = singles.tile([128, 128], bf16)
    nc.vector.tensor_copy(s.ident_h, ident)

    ones_f = singles.tile([128, 1], f32)
    nc.vector.memset(ones_f, 1.0)
    ones_rf = singles.tile([1, 128], f32)
    nc.vector.memset(ones_rf, 1.0)
    s.ones_col = singles.tile([128, 1], bf16)   # lhsT for denominator matmul
    nc.vector.tensor_copy(s.ones_col, ones_f)
    s.ones_row = singles.tile([1, 128], bf16)   # lhsT for broadcast matmuls
    nc.vector.tensor_copy(s.ones_row, ones_rf)

    _load(nc, s, 0)
    _prep(nc, s, 0)

    # Gate weights: Wtop = rows 0:128 of [w1|w2|w3], Wbot = rows 128:256.
    wstage = singles.tile([128, 2, 3, 128], f32)
    for gi, w in enumerate((w1, w2, w3)):
        nc.gpsimd.dma_start(out=wstage[:, 0, gi, :], in_=w[0:128, :])
        nc.gpsimd.dma_start(out=wstage[:, 1, gi, :], in_=w[128:256, :])
    s.w_top = singles.tile([128, 384], bf16)
    s.w_bot = singles.tile([128, 384], bf16)
    nc.vector.tensor_copy(s.w_top, wstage[:, 0, :, :])
    nc.vector.tensor_copy(s.w_bot, wstage[:, 1, :, :])

    bstage = singles.tile([1, 3, 128], f32)
    for gi, bvec in enumerate((b1, b2, b3)):
        nc.gpsimd.dma_start(out=bstage[:, gi, :],
                          in_=bvec.rearrange("(o p) -> o p", o=1))
    s.bcat = singles.tile([1, 384], bf16)
    nc.vector.tensor_copy(s.bcat, bstage)

    # Software pipeline: next batch's load/transpose work is emitted right
    # after this batch's score matmuls so the PE never starves at batch
    # boundaries.
    for bi in range(BPC):
        if bi + 1 < BPC:
            _load(nc, s, bi + 1)
        _phase_b(nc, s, bi)
        if bi + 1 < BPC:
            _prep(nc, s, bi + 1)
        _phase_c(nc, s, bi)
        _phase_d(nc, s, bi)


_NC_CACHE = {}


def _get_nc():
    if "nc" not in _NC_CACHE:
        nc = bacc.Bacc(None)
        with tile.TileContext(nc) as tc:
            with ExitStack() as ctx:
                _body(nc, tc, ctx)
        nc.finalize()
        _NC_CACHE["nc"] = nc
    return _NC_CACHE["nc"]


def _run(inputs, **kw):
    nc = _get_nc()
    in_maps = []
    for c in range(NCORES):
        m = {
            "P": np.ascontiguousarray(inputs["P"][c * BPC:(c + 1) * BPC]),
            "w_itr_att": np.asarray(inputs["w_itr_att"]),
            "w1": np.asarray(inputs["w1"]),
            "w2": np.asarray(inputs["w2"]),
            "w3": np.asarray(inputs["w3"]),
            "b1": np.asarray(inputs["b1"]),
            "b2": np.asarray(inputs["b2"]),
            "b3": np.asarray(inputs["b3"]),
        }
        in_maps.append({k: np.asarray(v, dtype=np.float32) for k, v in m.items()})
    res = run_bass_kernel_spmd(nc, in_maps, core_ids=list(range(NCORES)), **kw)
    outp = np.concatenate([r["out"] for r in res.results], axis=0)
    return outp.astype(np.float32), res


def kernel(**inputs):
    out, _ = _run(inputs)
    return out


# revision 19
# speedup vs baseline: 1.3530x; 1.0503x over previous
"""Trainium2 Bass kernel for nn_Encoding (dense transformer block with
inter-attention + gated fusion), data-parallel over batch on 8 NeuronCores.

Reference math per batch b (P: [n, d], weights small):
  wa, wb, wc = split(w_itr_att)
  A[i,j]   = P[i].wb + P[j].wa + sum_d P[i,d]*wc[d]*P[j,d]
  SA       = softmax_j(A)
  itr      = SA @ P
  Pc       = [P, itr]
  z = tanh(Pc@w1+b1); r = sig(Pc@w2+b2); f = sig(Pc@w3+b3)
  out      = r*P + f*z

Key tricks:
  - exp(P[i].wb) cancels between softmax numerator and denominator -> wb
    term dropped entirely.
  - Scores computed TRANSPOSED (At[j,i]) so P[j].wa is a per-partition
    exp() bias and both numerator (P^T @ T) and denominator (ones^T @ T)
    matmuls consume T=exp(At) in natural layout -- no [n,n] transpose.
  - Rows of P are processed in a fixed permutation (n = p*8+t instead of
    t*128+p) so each SBUF partition's DMA data is one contiguous 4KB run
    (128 descriptors per P load instead of 1024). Attention sums over all
    j and everything downstream uses the same permutation, so it cancels.
  - sigmoid(x) = 0.5 + 0.5*tanh(0.5*x) keeps all activations within the
    exp/tanh ACT table set (no table switches).
  - Matmuls run in bf16 (separate LDWEIGHTS pipelines behind MATMUL,
    unlike fp32/fp32r self-loading matmuls); accumulation is fp32 in
    PSUM; softmax/normalization arithmetic stays fp32.
"""
from contextlib import ExitStack

import numpy as np

import concourse.bass as bass
import concourse.mybir as mybir
import concourse.tile as tile
import concourse.tile_sem_assignment as tsa
from concourse import bacc
from concourse.bass_utils import run_bass_kernel_spmd
from concourse.masks import make_identity

# All HWDGE DMAs here are issued from the single SP sequencer (one physical
# FIFO ring -> in-order completion), so one completion semaphore suffices and
# keeps per-instruction sync-wait counts low.
tsa.NUM_HWDGE_SEMS = 1

B, N, D = 32, 1024, 128
NCORES = 8
BPC = B // NCORES          # batches per core
NB = N // 128              # 128-row blocks per batch
f32 = mybir.dt.float32
bf16 = mybir.dt.bfloat16
Exp = mybir.ActivationFunctionType.Exp
Tanh = mybir.ActivationFunctionType.Tanh


class _State:
    pass


def _load(nc, s, bi):
    """DMA P (permuted-contiguous rows: 128 x 4KB descriptors) + bf16 cast."""
    pn = s.work.tile([128, NB, 128], f32, tag="pn")
    nc.sync.dma_start(out=pn, in_=s.P[bi].rearrange("(p t) d -> p t d", t=NB))
    pn_h = s.work.tile([128, NB, 128], bf16, tag="pn_h")
    nc.vector.tensor_copy(pn_h, pn)
    s.pn[bi], s.pn_h[bi] = pn, pn_h


def _prep(nc, s, bi):
    """Build P^T (PE transpose), Pwc^T, and v = P.wa (DVE fused mul+reduce)."""
    work, ps_itr = s.work, s.ps_itr
    pn, pn_h = s.pn[bi], s.pn_h[bi]
    pt_h = work.tile([128, NB, 128], bf16, tag="pt_h")   # [d, n']
    for half in range(2):
        tp_ps = ps_itr.tile([128, 512], bf16, tag="itr")
        for q in range(4):
            jb = half * 4 + q
            nc.tensor.transpose(tp_ps[:, q * 128:(q + 1) * 128],
                                pn_h[:, jb, :], s.ident_h)
        nc.vector.tensor_copy(pt_h[:, half * 4:(half + 1) * 4, :], tp_ps)

    pwct_h = work.tile([128, NB, 128], bf16, tag="pwct_h")  # wc[d]*P^T
    nc.vector.tensor_scalar_mul(pwct_h, pt_h, s.wc_col)

    # v[j] = P[j].wa on the DVE (keeps PE and the shared PSUM rings free)
    v_sb = work.tile([128, NB], f32, tag="v_sb")
    vscr = work.tile([128, 128], f32, tag="vscr")
    for jb in range(NB):
        nc.vector.tensor_mul(vscr, pn[:, jb, :], s.wa_b)
        nc.vector.reduce_sum(v_sb[:, jb:jb + 1], vscr,
                             axis=mybir.AxisListType.X)
    s.pt_h[bi], s.pwct_h[bi], s.v_sb[bi] = pt_h, pwct_h, v_sb


def _phase_b(nc, s, bi):
    """Scores At[j,i] + exp -> T (bf16)."""
    st = s.big.tile([128, NB, N], bf16, tag="st")
    pt_h, pwct_h, v_sb = s.pt_h[bi], s.pwct_h[bi], s.v_sb[bi]
    for jb in range(NB):
        at_ps = s.ps_at.tile([128, 1024], f32, tag="at")
        nc.tensor.matmul(at_ps[:, 0:512], pt_h[:, jb, :],
                         pwct_h[:, 0:4, :], start=True, stop=True)
        nc.tensor.matmul(at_ps[:, 512:1024], pt_h[:, jb, :],
                         pwct_h[:, 4:8, :], start=True, stop=True)
        nc.scalar.activation(st[:, jb, :], at_ps, Exp, bias=v_sb[:, jb:jb + 1])
    s.st[bi] = st


def _phase_c(nc, s, bi):
    """numerator/denominator matmuls + normalization -> itr^T (bf16)."""
    work, ps_itr, ps_dv = s.work, s.ps_itr, s.ps_dv
    st, pn_h = s.st[bi], s.pn_h[bi]
    itrt_h = work.tile([128, NB, 128], bf16, tag="itrt_h")  # itr^T [d, n']
    for c in range(2):
        cs = slice(c * 512, (c + 1) * 512)
        itr_ps = ps_itr.tile([128, 512], f32, tag="itr")
        den_ps = ps_dv.tile([1, 512], f32, tag="dv")
        for jb in range(NB):
            nc.tensor.matmul(den_ps, s.ones_col, st[:, jb, cs],
                             start=(jb == 0), stop=(jb == NB - 1))
        # broadcast raw denominator to all partitions via ones x den, then
        # fast reciprocal on [128,512] (lane-parallel) and multiply. The
        # two PSUM->SBUF moves ride on the scalar engine to keep the DVE
        # queue off this PE-blocking chain.
        den_row = work.tile([1, 512], bf16, tag="den_row")
        nc.scalar.copy(den_row, den_ps)
        bc_ps = ps_dv.tile([128, 512], f32, tag="dv")
        nc.tensor.matmul(bc_ps, s.ones_row, den_row, start=True, stop=True)
        bc_sb = work.tile([128, 512], f32, tag="bc_sb")
        nc.scalar.copy(bc_sb, bc_ps)
        recip_sb = work.tile([128, 512], f32, tag="recip_sb")
        nc.vector.reciprocal_approx_fast(recip_sb, bc_sb)
        for jb in range(NB):
            nc.tensor.matmul(itr_ps, pn_h[:, jb, :], st[:, jb, cs],
                             start=(jb == 0), stop=(jb == NB - 1))
        with nc.allow_low_precision(reason="bf16 itr weights"):
            nc.vector.tensor_mul(itrt_h[:, c * 4:(c + 1) * 4, :],
                                 itr_ps, recip_sb)
    s.itrt_h[bi] = itrt_h


def _phase_d(nc, s, bi):
    """Gates + output, in two half-batches so ACT/DVE/DMA pipeline."""
    work, ps_dv = s.work, s.ps_dv
    pn, pt_h, itrt_h = s.pn[bi], s.pt_h[bi], s.itrt_h[bi]
    out_sb = work.tile([128, NB, 128], f32, tag="out_sb")
    for half in range(2):
        hb = slice(half * 4, (half + 1) * 4)
        gcat = work.tile([128, 4, 384], f32, tag="gcat")
        for q in range(4):
            ib = half * 4 + q
            g_ps = ps_dv.tile([128, 384], f32, tag="dv")
            nc.tensor.matmul(g_ps, pt_h[:, ib, :], s.w_top,
                             start=True, stop=False)
            nc.tensor.matmul(g_ps, itrt_h[:, ib, :], s.w_bot,
                             start=False, stop=True)
            nc.vector.tensor_add(gcat[:, q, :], g_ps, s.bias_bc)

        z_t = work.tile([128, 4, 128], f32, tag="z_t")
        nc.scalar.activation(z_t, gcat[:, :, 0:128], Tanh)
        rf_t = work.tile([128, 4, 256], f32, tag="rf_t")
        nc.scalar.activation(rf_t, gcat[:, :, 128:384], Tanh, scale=0.5)
        # r = 0.5 + 0.5*tanh(0.5 x), f likewise
        rf_a = work.tile([128, 4, 256], f32, tag="rf_a")
        eng = nc.vector if bi == BPC - 1 else nc.gpsimd
        eng.tensor_scalar(rf_a, rf_t, 0.5, 0.5,
                          mybir.AluOpType.mult, mybir.AluOpType.add)
        m1 = work.tile([128, 4, 128], f32, tag="m1")
        eng.tensor_mul(m1, rf_a[:, :, 0:128], pn[:, hb, :])         # r * P
        m2 = work.tile([128, 4, 128], f32, tag="m2")
        nc.vector.tensor_mul(m2, rf_a[:, :, 128:256], z_t)          # f * z
        nc.vector.tensor_add(out_sb[:, hb, :], m1, m2)

    nc.sync.dma_start(out=s.out[bi].rearrange("(p t) d -> p t d", t=NB),
                      in_=out_sb)


def _body(nc, tc, ctx):
    s = _State()
    s.P = nc.dram_tensor("P", [BPC, N, D], f32, kind="ExternalInput")
    w_att = nc.dram_tensor("w_itr_att", [3 * D], f32, kind="ExternalInput")
    w1 = nc.dram_tensor("w1", [2 * D, D], f32, kind="ExternalInput")
    w2 = nc.dram_tensor("w2", [2 * D, D], f32, kind="ExternalInput")
    w3 = nc.dram_tensor("w3", [2 * D, D], f32, kind="ExternalInput")
    b1 = nc.dram_tensor("b1", [D], f32, kind="ExternalInput")
    b2 = nc.dram_tensor("b2", [D], f32, kind="ExternalInput")
    b3 = nc.dram_tensor("b3", [D], f32, kind="ExternalInput")
    s.out = nc.dram_tensor("out", [BPC, N, D], f32, kind="ExternalOutput")

    singles = ctx.enter_context(tc.tile_pool(name="singles", bufs=1))
    s.work = ctx.enter_context(tc.tile_pool(name="work", bufs=2))
    s.big = ctx.enter_context(tc.tile_pool(name="big", bufs=2))
    s.ps_at = ctx.enter_context(tc.tile_pool(name="ps_at", bufs=2, space="PSUM"))
    s.ps_itr = ctx.enter_context(tc.tile_pool(name="ps_itr", bufs=2, space="PSUM"))
    s.ps_dv = ctx.enter_context(tc.tile_pool(name="ps_dv", bufs=2, space="PSUM"))
    s.pn, s.pn_h, s.pt_h, s.pwct_h = {}, {}, {}, {}
    s.v_sb, s.st, s.itrt_h = {}, {}, {}

    # ---- constants ----
    # w_itr_att as a single-descriptor row; wa/wc become per-partition
    # columns via tiny K=1 fp32 matmuls (exact: multiply by 1.0).
    watt_row = singles.tile([1, 3 * D], f32)
    nc.sync.dma_start(out=watt_row, in_=w_att.rearrange("(o c) -> o c", o=1))
    ones2_f = singles.tile([1, 2], f32)
    nc.vector.memset(ones2_f, 1.0)
    wcols_ps = s.ps_dv.tile([128, 2], f32, tag="dv")
    nc.tensor.matmul(wcols_ps, watt_row[:, 256:384], ones2_f,
                     start=True, stop=True)          # wc
    s.wc_col = singles.tile([128, 1], f32)
    nc.vector.tensor_copy(s.wc_col, wcols_ps[:, 0:1])
    # wa replicated across all partitions via ones x wa_row (exact: x1.0)
    ones_rf0 = singles.tile([1, 128], f32)
    nc.vector.memset(ones_rf0, 1.0)
    wab_ps = s.ps_at.tile([128, 512], f32, tag="at")
    nc.tensor.matmul(wab_ps[:, 0:128], ones_rf0, watt_row[:, 0:128],
                     start=True, stop=True)
    s.wa_b = singles.tile([128, 128], f32)
    nc.vector.tensor_copy(s.wa_b, wab_ps[:, 0:128])

    ident = singles.tile([128, 128], f32)
    make_identity(nc, ident)
    s.ident_h = singles.tile([128, 128], bf16)
    nc.vector.tensor_copy(s.ident_h, ident)

    ones_f = singles.tile([128, 1], f32)
    nc.vector.memset(ones_f, 1.0)
    ones_rf = singles.tile([1, 128], f32)
    nc.vector.memset(ones_rf, 1.0)
    s.ones_col = singles.tile([128, 1], bf16)   # lhsT for denominator matmul
    nc.vector.tensor_copy(s.ones_col, ones_f)
    s.ones_row = singles.tile([1, 128], bf16)   # lhsT for broadcast matmuls
    nc.vector.tensor_copy(s.ones_row, ones_rf)

    _load(nc, s, 0)
    _prep(nc, s, 0)

    # Gate weights: Wtop = rows 0:128 of [w1|w2|w3], Wbot = rows 128:256.
    wstage = singles.tile([128, 2, 3, 128], f32)
    for gi, w in enumerate((w1, w2, w3)):
        nc.gpsimd.dma_start(out=wstage[:, 0, gi, :], in_=w[0:128, :])
        nc.gpsimd.dma_start(out=wstage[:, 1, gi, :], in_=w[128:256, :])
    s.w_top = singles.tile([128, 384], bf16)
    s.w_bot = singles.tile([128, 384], bf16)
    nc.vector.tensor_copy(s.w_top, wstage[:, 0, :, :])
    nc.vector.tensor_copy(s.w_bot, wstage[:, 1, :, :])

    bstage = singles.tile([1, 3, 128], f32)
    for gi, bvec in enumerate((b1, b2, b3)):
        nc.gpsimd.dma_start(out=bstage[:, gi, :],
                          in_=bvec.rearrange("(o p) -> o p", o=1))
    s.bcat = singles.tile([1, 384], bf16)
    nc.vector.tensor_copy(s.bcat, bstage)
    bias_ps = s.ps_dv.tile([128, 384], f32, tag="dv")
    nc.tensor.matmul(bias_ps, s.ones_row, s.bcat, start=True, stop=True)
    s.bias_bc = singles.tile([128, 384], f32)
    nc.vector.tensor_copy(s.bias_bc, bias_ps)

    # Software pipeline: next batch's load/transpose work is emitted right
    # after this batch's score matmuls so the PE never starves at batch
    # boundaries.
    for bi in range(BPC):
        if bi + 1 < BPC:
            _load(nc, s, bi + 1)
        _phase_b(nc, s, bi)
        if bi + 1 < BPC:
            _prep(nc, s, bi + 1)
        _phase_c(nc, s, bi)
        _phase_d(nc, s, bi)


_NC_CACHE = {}


def _get_nc():
    if "nc" not in _NC_CACHE:
        nc = bacc.Bacc(None)
        with tile.TileContext(nc) as tc:
            with ExitStack() as ctx:
                _body(nc, tc, ctx)
        nc.finalize()
        _NC_CACHE["nc"] = nc
    return _NC_CACHE["nc"]


def _run(inputs, **kw):
    nc = _get_nc()
    in_maps = []
    for c in range(NCORES):
        m = {
            "P": np.ascontiguousarray(inputs["P"][c * BPC:(c + 1) * BPC]),
            "w_itr_att": np.asarray(inputs["w_itr_att"]),
            "w1": np.asarray(inputs["w1"]),
            "w2": np.asarray(inputs["w2"]),
            "w3": np.asarray(inputs["w3"]),
            "b1": np.asarray(inputs["b1"]),
            "b2": np.asarray(inputs["b2"]),
            "b3": np.asarray(inputs["b3"]),
        }
        in_maps.append({k: np.asarray(v, dtype=np.float32) for k, v in m.items()})
    res = run_bass_kernel_spmd(nc, in_maps, core_ids=list(range(NCORES)), **kw)
    outp = np.concatenate([r["out"] for r in res.results], axis=0)
    return outp.astype(np.float32), res


def kernel(**inputs):
    out, _ = _run(inputs)
    return out
